# revision 1
# baseline (speedup 1.0000x reference)
"""AdderVDSR kernel for 8 TRN2 NeuronCores.

Mathematical collapse: every AdderNet block computes
    relu(-sum_{c,kh,kw} |patch - w|)
and the inner sum of 576 absolute values of continuous random quantities is
strictly positive, so each block outputs exactly 0 in fp32.  After the first
adder layer the hidden state is identically zero and stays zero, so

    reference(x, ...) == pixel_shuffle(conv3(x, up_w, up_b), 2) + out_b

bit-exactly (conv3 of a zero tensor is exactly zero; relu of a negative
number is exactly 0.0).  The kernel therefore only computes the 3->12 channel
3x3 up-conv, the pixel shuffle, and the two bias adds.

Distribution: data-parallel over H.  Core i computes pre-shuffle rows
[16*i, 16*i+16) -> output rows [32*i, 32*i+32).  The host shards x into
per-core im2col patch tensors in bf16 (layout replication only; all FLOPs
run on device; fp32 PSUM accumulate keeps rel err ~2.5e-3).  On device: one
bf16 matmul per (row-half, parity, batch, bank-pair); biases fold in via a
ones row; the pixel-shuffle column interleave happens in the PSUM->SBUF
stage (stride-2 destinations, VectorE b=0 / ScalarE b=1) pipelined behind
the matmuls at half-slab granularity; f32 output leaves over three DMA
paths (SP ring / ACT ring / Pool SWDGE) as 12 half-slab transfers.
"""

import numpy as np

import concourse.bass as bass
import concourse.mybir as mybir
from concourse.bass_utils import run_bass_kernel_spmd

N_CORES = 8
B, C, H, W = 2, 3, 128, 128
RH = H // N_CORES          # 16 pre-shuffle rows per core
NPIX = B * RH * W          # 4096 pre-shuffle pixels per core
OC = 12                    # up-conv output channels (= 4*C)
K = 28                     # im2col contraction: 27 taps + ones row (bias)
XW = NPIX + 16             # xcol width: patches + packed weight columns

_f32 = mybir.dt.float32
_bf16 = mybir.dt.bfloat16


def build_graph():
    nc = bass.Bass()
    xcol = nc.declare_dram_parameter("xcol", [K, XW], _bf16, isOutput=False)
    out = nc.declare_dram_parameter("out", [B, C, 2 * RH, 2 * W], _f32, isOutput=True)

    with (
        nc.sbuf_tensor([K, XW], _bf16) as P,
        nc.sbuf_tensor([38, NPIX + 16], _f32) as sb_out,
        nc.psum_tensor([38, NPIX // 4], _f32) as pse_h0,
        nc.psum_tensor([38, NPIX // 4], _f32) as pso_h0,
        nc.psum_tensor([38, NPIX // 4], _f32) as pse_h1,
        nc.psum_tensor([38, NPIX // 4], _f32) as pso_h1,
        nc.semaphore("dma_in") as dma_in,
        nc.semaphore("dma_in2") as dma_in2,
        nc.semaphore("dma_in3") as dma_in3,
        nc.semaphore("mm_sem") as mm_sem,
        nc.semaphore("cp0") as cp0,
        nc.semaphore("cp1") as cp1,
        nc.semaphore("dma_out_sem") as dma_out_sem,
        nc.semaphore("dma_out2_sem") as dma_out2_sem,
        nc.Block() as block,
    ):
        # xcol column layout: [wb (16) | b0h0 | b1h0 | b0h1 | b1h1] (1024 each)
        def wslice(dc):
            return P[:, 6 * dc : 6 * dc + 6]

        def rhslice(b, rq):
            h, r2 = divmod(rq, 2)
            lo = 16 + h * 2048 + b * 1024 + r2 * 512
            return P[0:K, lo : lo + 512]

        def out_dma(eng, b, c, h, sem):
            # Half-slab h covers pre-shuffle rows [8h, 8h+8) -> output rows
            # [16h, 16h+16).
            src = sb_out[
                32 * b + 2 * c : 32 * b + 2 * c + 2, 2048 * h : 2048 * (h + 1)
            ].rearrange("dr (r col) -> dr r col", r=RH // 2, col=2 * W)
            dst = out[b, c, 16 * h : 16 * (h + 1), :].rearrange(
                "(r dr) col -> dr r col", dr=2
            )
            return eng.dma_start(out=dst, in_=src).then_inc(sem, 16)

        @block.sync
        def _(sync):
            # Input in 3 chunks ordered by first use: the h0/b0 matmuls start
            # after a small first chunk while later chunks' completion
            # latency hides behind compute: [w + b0h0 | b1h0 | h1].
            sync.dma_start(out=P[:, :1040], in_=xcol[:, :1040]).then_inc(dma_in, 16)
            sync.dma_start(out=P[:, 1040:2064], in_=xcol[:, 1040:2064]).then_inc(
                dma_in2, 16
            )
            sync.dma_start(out=P[:, 2064:], in_=xcol[:, 2064:]).then_inc(dma_in3, 16)
            # Half-slab output DMAs spread over three issuers (each dma_start
            # occupies its issuing engine for the whole transfer).  ACT is
            # copying until ~h1 time, so SP and Pool take all six h0 slabs
            # (3 each) and the six h1 slabs go 2/2/2.
            sync.wait_ge(cp1, 1)
            for c in range(C):
                out_dma(sync, 1, c, 0, dma_out_sem)
            sync.wait_ge(cp1, 2)
            out_dma(sync, 1, 0, 1, dma_out_sem)
            sync.wait_ge(cp0, 2)
            for c in range(C):
                out_dma(sync, 0, c, 1, dma_out_sem)
            sync.wait_ge(dma_out_sem, 144)
            sync.wait_ge(dma_out2_sem, 48)

        @block.gpsimd
        def _(gpsimd):
            # Pool takes ONLY early b=0 h0 slabs: its end-of-block SWDGE
            # drain (~3.5us) then overlaps SP/ACT's late h1 transfers.
            gpsimd.wait_ge(cp0, 1)
            for c in range(C):
                out_dma(gpsimd, 0, c, 0, dma_out2_sem)

        @block.tensor
        def _(tensor):
            # Half-outer, parity-next order: copies of quad (h, dc) start as
            # soon as its 4 matmuls retire, while the PE moves on to other
            # quads.  Each quad owns its own PSUM bank pair, so a PE write
            # never shares a bank with a concurrent DVE/ACT read.
            # Partition 32*b + (c*2+dr); slot (rq-2h)*512 + r*W + col.
            quads = ((pse_h0, pso_h0), (pse_h1, pso_h1))
            for h in range(2):
                for dc in range(2):
                    pst = quads[h][dc]
                    for b in range(B):
                        pb = 32 * b
                        for rq in (2 * h, 2 * h + 1):
                            if dc == 0 and rq == 2 * h:
                                if h == 0:
                                    tensor.wait_ge(dma_in if b == 0 else dma_in2, 16)
                                elif b == 0:
                                    tensor.wait_ge(dma_in3, 16)
                            o = pst[pb : pb + 6, (rq - 2 * h) * 512 : (rq - 2 * h + 1) * 512]
                            mm = tensor.matmul(
                                o, lhsT=wslice(dc), rhs=rhslice(b, rq),
                                start=True, stop=True,
                            )
                    mm.then_inc(mm_sem, 1)

        # PSUM -> SBUF staging with the pixel-shuffle column interleave
        # (stride-2 destinations).  Lane-aligned; VectorE takes b=0 while
        # ScalarE takes b=1.
        @block.vector
        def _(vector):
            for h in range(2):
                for dc in range(2):
                    pst = ((pse_h0, pso_h0), (pse_h1, pso_h1))[h][dc]
                    vector.wait_ge(mm_sem, 2 * h + dc + 1)
                    cp = vector.tensor_copy(
                        sb_out[0:6, 2048 * h + dc : 2048 * (h + 1) : 2], pst[0:6, :]
                    )
                    if dc == 1:
                        cp.then_inc(cp0, 1)

        @block.scalar
        def _(scalar):
            # Dummy tiny copy: pulls the ACT_TABLE_LOAD for Copy forward,
            # off the post-matmul critical path.
            scalar.wait_ge(dma_in, 16)
            scalar.copy(sb_out[32:33, NPIX : NPIX + 16], P[0:1, 0:16])
            for h in range(2):
                for dc in range(2):
                    pst = ((pse_h0, pso_h0), (pse_h1, pso_h1))[h][dc]
                    scalar.wait_ge(mm_sem, 2 * h + dc + 1)
                    cp = scalar.copy(
                        sb_out[32:38, 2048 * h + dc : 2048 * (h + 1) : 2], pst[32:38, :]
                    )
                    if dc == 1:
                        cp.then_inc(cp1, 1)
            # Two b=1 h1 slabs on the ACT HWDGE ring after ALL copies (a
            # mid-stream DMA would stall the h1 copies for its whole
            # transfer).  Self-wait: the DMA must not read sb_out before the
            # deep ACT pipeline has retired the copies.
            scalar.wait_ge(cp1, 2)
            out_dma(scalar, 1, 1, 1, dma_out_sem)
            out_dma(scalar, 1, 2, 1, dma_out_sem)

    return nc


def make_in_maps(x, up_w, up_b, out_b):
    """Shard inputs: per-core im2col patches with packed weight columns."""
    import ml_dtypes

    bf16 = ml_dtypes.bfloat16
    x = np.asarray(x, dtype=np.float32)
    up_w = np.asarray(up_w, dtype=np.float32)
    up_b = np.asarray(up_b, dtype=np.float32)
    out_b = np.asarray(out_b, dtype=np.float32)

    # wb[c2*9+kh*3+kw, 6*dc + (c*2+dr)] = up_w[c*4+dr*2+dc, c2, kh, kw]
    # wb[27, 6*dc + (c*2+dr)] = up_b[o] + out_b[c]
    wb = np.zeros((K, 16), dtype=np.float32)
    for c in range(C):
        for dr in range(2):
            for dc in range(2):
                o = c * 4 + dr * 2 + dc
                col = 6 * dc + c * 2 + dr
                wb[:27, col] = up_w[o].reshape(27)
                wb[27, col] = up_b[o] + out_b[c]

    xp = np.zeros((B, C, H + 2, W + 2), dtype=np.float32)
    xp[:, :, 1 : H + 1, 1 : W + 1] = x

    in_maps = []
    for i in range(N_CORES):
        xcol = np.empty((K, XW), dtype=np.float32)
        # columns 16: onward as [h, b, r2 (8 rows), W]
        pat = xcol[:, 16:].reshape(K, 2, B, RH // 2, W)
        for c in range(C):
            for kh in range(3):
                for kw in range(3):
                    k = c * 9 + kh * 3 + kw
                    for h in range(2):
                        r0 = 16 * i + 8 * h + kh
                        pat[k, h] = xp[:, c, r0 : r0 + RH // 2, kw : kw + W]
        pat[27] = 1.0
        xcol[:, :16] = wb
        in_maps.append({"xcol": xcol.astype(bf16)})
    return in_maps


def kernel(x, up_w, up_b, in_w, in_b, adder_w, out_w, out_b):
    nc = build_graph()
    in_maps = make_in_maps(x, up_w, up_b, out_b)
    res = run_bass_kernel_spmd(nc, in_maps, core_ids=list(range(N_CORES)))
    slabs = [np.asarray(res.results[i]["out"]) for i in range(N_CORES)]
    return np.concatenate(slabs, axis=2).astype(np.float32)



# revision 5
# speedup vs baseline: 1.3346x; 1.3346x over previous
"""AdderVDSR kernel for 8 TRN2 NeuronCores.

Mathematical collapse: every AdderNet block computes
    relu(-sum_{c,kh,kw} |patch - w|)
and the inner sum of 576 absolute values of continuous random quantities is
strictly positive, so each block outputs exactly 0 in fp32.  After the first
adder layer the hidden state is identically zero and stays zero, so

    reference(x, ...) == pixel_shuffle(conv3(x, up_w, up_b), 2) + out_b

bit-exactly.  The kernel therefore only computes the 3->12 channel 3x3
up-conv, the pixel shuffle, and the two bias adds.

Distribution: data-parallel over H; core i computes pre-shuffle rows
[16i, 16i+16) -> output rows [32i, 32i+32).

Device formulation (the baseline's weakness was 2-partition output DMAs
that landed on only 2 of 16 DMA engines, 6-lane copies, and 16 small
matmuls): a block-diagonal im2col GEMM over G=8 groups (b x four 4-row
blocks).  K = 8 groups x 14 taps = 112 contracted over two accumulating
matmul passes (taps 0..13, then 14..26 plus a ones/bias row), M = 128 PSUM
partitions = 4 shuffle phases (dr,dc) at 32-aligned bases (+ 8c + g, top 8
of each quadrant zero), N = 512 pixels
(4 rows x 128 w) -- one PSUM bank.  Four pixel-shuffle interleave copies
(one per (dr,dc) phase, 24 lanes each, split DVE/ACT) land in a 24-partition
SBUF layout where each partition is an 8 KiB DRAM-contiguous slab row, so
the two output DMAs fan out across the DMA engines.
"""

import numpy as np

import concourse.bass as bass
import concourse.mybir as mybir
from concourse.bass_utils import run_bass_kernel_spmd

N_CORES = 8
B, C, H, W = 2, 3, 128, 128
RH = H // N_CORES          # 16 pre-shuffle rows per core
G = 8                      # groups: (b, 4-row block)
KP = 14                    # taps per pass (27 taps + ones row = 2x14)
KK = G * KP                # 112 matmul contraction rows
M = 128                    # psum partitions: 32*(dr,dc) + 8c + g, 32-aligned
NW = 512                   # pixels per group: 4 rows x 128 w
WCOLS = 2 * M              # two passes' stationary weights
XW = WCOLS + 2 * NW        # 1216 total xcol columns
SPLIT = WCOLS + NW         # input chunk boundary (weights + pass-0 rhs)

_f32 = mybir.dt.float32
_bf16 = mybir.dt.bfloat16


def build_graph():
    nc = bass.Bass()
    xcol = nc.declare_dram_parameter("xcol", [KK, XW], _bf16, isOutput=False)
    out = nc.declare_dram_parameter("out", [B, C, 2 * RH, 2 * W], _f32, isOutput=True)

    with (
        nc.sbuf_tensor([KK, XW], _bf16) as P,
        nc.sbuf_tensor([24, 2048], _f32) as sb_out,
        nc.sbuf_tensor([1, 16], _bf16) as scratch,
        nc.psum_tensor([M, NW], _f32) as pst,
        nc.semaphore("dma_a") as dma_a,
        nc.semaphore("dma_b") as dma_b,
        nc.semaphore("mm_sem") as mm_sem,
        nc.semaphore("cp_v") as cp_v,
        nc.semaphore("cp_s") as cp_s,
        nc.semaphore("out_sem") as out_sem,
        nc.Block() as block,
    ):
        # Output DRAM view: partition q = 12b + 4c + rb holds the contiguous
        # 8 KiB slab out[b, c, 8*rb : 8*rb+8, :].
        out_v = out.rearrange("b c (rb rows) w -> (b c rb) (rows w)", rb=4)

        def copy_ap(e):
            dr, dc = e // 2, e % 2
            src = pst[32 * e : 32 * e + 24, :].rearrange("p (n w) -> p n w", n=4)
            dst = sb_out.rearrange(
                "q (n dr w dc) -> q n dr w dc", n=4, dr=2, w=128, dc=2
            )[:, :, dr, :, dc]
            return dst, src

        @block.sync
        def _(sync):
            # Input in 2 chunks: matmul pass 0 starts after [weights | rhs0].
            sync.dma_start(out=P[:, :SPLIT], in_=xcol[:, :SPLIT]).then_inc(dma_a, 16)
            sync.dma_start(out=P[:, SPLIT:], in_=xcol[:, SPLIT:]).then_inc(dma_b, 16)
            # First half of the output as soon as all 4 interleave copies land.
            sync.wait_ge(cp_v, 2)
            sync.wait_ge(cp_s, 2)
            sync.dma_start(out=out_v[0:12], in_=sb_out[0:12, :]).then_inc(out_sem, 16)
            sync.wait_ge(out_sem, 32)

        @block.tensor
        def _(tensor):
            tensor.wait_ge(dma_a, 16)
            tensor.matmul(
                pst[:, :], lhsT=P[:, 0:M], rhs=P[:, WCOLS:SPLIT],
                start=True, stop=False,
            )
            tensor.wait_ge(dma_b, 16)
            tensor.matmul(
                pst[:, :], lhsT=P[:, M:WCOLS], rhs=P[:, SPLIT:XW],
                start=False, stop=True,
            ).then_inc(mm_sem, 1)

        @block.vector
        def _(vector):
            vector.wait_ge(mm_sem, 1)
            for e in (0, 1):
                dst, src = copy_ap(e)
                vector.tensor_copy(dst, src).then_inc(cp_v, 1)

        @block.scalar
        def _(scalar):
            # Dummy tiny copy pulls ACT_TABLE_LOAD off the critical path.
            scalar.copy(scratch[0:1, 0:16], P[0:1, 0:16])
            scalar.wait_ge(mm_sem, 1)
            for e in (2, 3):
                dst, src = copy_ap(e)
                scalar.copy(dst, src).then_inc(cp_s, 1)
            scalar.wait_ge(cp_v, 2)
            scalar.dma_start(out=out_v[12:24], in_=sb_out[12:24, :]).then_inc(
                out_sem, 16
            )

    return nc


def make_in_maps(x, up_w, up_b, out_b):
    """Shard inputs: per-core block-diagonal im2col + packed weights."""
    import ml_dtypes

    bf16 = ml_dtypes.bfloat16
    x = np.asarray(x, dtype=np.float32)
    up_w = np.asarray(up_w, dtype=np.float32)
    up_b = np.asarray(up_b, dtype=np.float32)
    out_b = np.asarray(out_b, dtype=np.float32)

    xp = np.zeros((B, C, H + 2, W + 2), dtype=np.float32)
    xp[:, :, 1 : H + 1, 1 : W + 1] = x

    # Stationary weights, shared across cores.
    # lhsT_t[14g + kappa, m] = [g == m%8] * w(tau=14t+kappa; m), block-diagonal.
    wb = np.zeros((KK, WCOLS), dtype=np.float32)
    for e in range(4):
        dr, dc = e // 2, e % 2
        for c in range(C):
            o = c * 4 + dr * 2 + dc
            for g in range(G):
                b2, rb = divmod(g, 4)
                m = e * 32 + 12 * b2 + 4 * c + rb
                for tau in range(27):
                    c2, kh, kw = tau // 9, (tau // 3) % 3, tau % 3
                    t, kappa = divmod(tau, KP)
                    wb[KP * g + kappa, M * t + m] = up_w[o, c2, kh, kw]
                # tau=27 (t=1, kappa=13): ones-row bias
                wb[KP * g + 13, M + m] = up_b[o] + out_b[c]

    in_maps = []
    for i in range(N_CORES):
        xcol = np.empty((KK, XW), dtype=np.float32)
        xcol[:, :WCOLS] = wb
        pat = xcol[:, WCOLS:].reshape(KK, 2, 4, W)  # [row, t, n, w]
        for g in range(G):
            b, rb = divmod(g, 4)
            r0 = RH * i + 4 * rb
            for kappa in range(KP):
                for t in range(2):
                    tau = KP * t + kappa
                    if tau == 27:
                        pat[KP * g + kappa, t] = 1.0
                    else:
                        c, kh, kw = tau // 9, (tau // 3) % 3, tau % 3
                        pat[KP * g + kappa, t] = xp[
                            b, c, r0 + kh : r0 + kh + 4, kw : kw + W
                        ]
        in_maps.append({"xcol": xcol.astype(bf16)})
    return in_maps


def kernel(x, up_w, up_b, in_w, in_b, adder_w, out_w, out_b):
    nc = build_graph()
    in_maps = make_in_maps(x, up_w, up_b, out_b)
    res = run_bass_kernel_spmd(nc, in_maps, core_ids=list(range(N_CORES)))
    slabs = [np.asarray(res.results[i]["out"]) for i in range(N_CORES)]
    return np.concatenate(slabs, axis=2).astype(np.float32)


# revision 13
# speedup vs baseline: 1.4574x; 1.0920x over previous
"""AdderVDSR kernel for 8 TRN2 NeuronCores.

Mathematical collapse: every AdderNet block computes
    relu(-sum_{c,kh,kw} |patch - w|)
and the inner sum of 576 absolute values of continuous random quantities is
strictly positive, so each block outputs exactly 0 in fp32.  After the first
adder layer the hidden state is identically zero and stays zero, so

    reference(x, ...) == pixel_shuffle(conv3(x, up_w, up_b), 2) + out_b

bit-exactly.  The kernel therefore only computes the 3->12 channel 3x3
up-conv, the pixel shuffle, and the two bias adds.

Distribution: data-parallel over H; core i computes pre-shuffle rows
[16i, 16i+16) -> output rows [32i, 32i+32).

Device formulation: block-diagonal im2col GEMM over G=8 groups (b x four
4-row blocks).  K = 8 groups x 14 taps = 112, contracted in two accumulating
passes (taps 0..13, then 14..26 + ones/bias row); M = 128 PSUM partitions =
4 shuffle phases (dr,dc) at 32-aligned bases + 8c + g; N = 512 pixels in one
PSUM bank, pipelined as two 256-column halves (4 matmuls).  Pixel-shuffle
interleave copies (24 lanes, phase x col-half, split DVE/ACT) land in a
24-partition SBUF layout whose partitions are 8 KiB DRAM-contiguous slab
rows, so each output DMA fans out across all DMA engines.

The NEFF epilogue (walrus' ~7us all-semaphore reset sweep, gated on the
block-exit barrier) dominates at this scale, so no engine waits for output
DMA completion: the sweep overlaps the output drain.
"""

import numpy as np

import concourse.bass as bass
import concourse.mybir as mybir
from concourse.bass_utils import run_bass_kernel_spmd

N_CORES = 8
B, C, H, W = 2, 3, 128, 128
RH = H // N_CORES          # 16 pre-shuffle rows per core
G = 8                      # groups: (b, 4-row block)
KP = 14                    # taps per pass (27 taps + ones row = 2x14)
KK = G * KP                # 112 matmul contraction rows
M = 128                    # psum partitions: 32*(dr,dc) + 8c + g, 32-aligned
NW = 512                   # pixels per group: 4 rows x 128 w
NH = NW // 2               # matmul column-half
WCOLS = 2 * M              # two passes' stationary weights
XW = WCOLS + 2 * NW        # 1280 total xcol columns

_f32 = mybir.dt.float32
_bf16 = mybir.dt.bfloat16

# rhs column blocks: [weights | p0c0 | p0c1 | p1c0 | p1c1]
P0 = WCOLS
P1 = WCOLS + NW
# input chunks (A covers weights + p0c0, B covers p0c1 + p1c0, C covers p1c1)
CHA = WCOLS + NH
CHB = WCOLS + NW + NH


def build_graph():
    nc = bass.Bass()
    xcol = nc.declare_dram_parameter("xcol", [KK, XW], _bf16, isOutput=False)
    out = nc.declare_dram_parameter("out", [B, C, 2 * RH, 2 * W], _f32, isOutput=True)

    with (
        nc.sbuf_tensor([KK, XW], _bf16) as P,
        nc.sbuf_tensor([24, 2048], _f32) as sb_out,
        nc.sbuf_tensor([1, 16], _bf16) as scratch,
        nc.psum_tensor([M, 2 * NW], _f32) as pst,
        nc.semaphore("dma_a") as dma_a,
        nc.semaphore("dma_b") as dma_b,
        nc.semaphore("dma_c") as dma_c,
        nc.semaphore("mm_sem") as mm_sem,
        nc.semaphore("cp_v") as cp_v,
        nc.semaphore("cp_s") as cp_s,
        nc.semaphore("out_sem") as out_sem,
        nc.Block() as block,
    ):
        # Output DRAM view: partition q = 12b + 4c + rb is the contiguous
        # 8 KiB slab out[b, c, 8*rb : 8*rb+8, :]; h splits it into 4 KiB
        # row-halves matching psum column-halves.
        out_v = out.rearrange(
            "b c (rb h rows) w -> (b c rb) h (rows w)", rb=4, h=2, rows=4
        )

        def copy_ap(e, h):
            dr, dc = e // 2, e % 2
            src = pst[32 * e : 32 * e + 24, NW * h : NW * h + NH].rearrange(
                "p (n w) -> p n w", n=2
            )
            dst = sb_out.rearrange(
                "q (n dr w dc) -> q n dr w dc", n=4, dr=2, w=128, dc=2
            )[:, 2 * h : 2 * h + 2, dr, :, dc]
            return dst, src

        @block.sync
        def _(sync):
            sync.dma_start(out=P[:, :CHA], in_=xcol[:, :CHA]).then_inc(dma_a, 16)
            sync.dma_start(out=P[:, CHA:CHB], in_=xcol[:, CHA:CHB]).then_inc(dma_b, 16)
            sync.dma_start(out=P[:, CHB:], in_=xcol[:, CHB:]).then_inc(dma_c, 16)
            # Row-half output DMAs chase the interleave copies; nothing waits
            # for their completion -- the NEFF epilogue overlaps the drain.
            sync.wait_ge(cp_v, 2)
            sync.wait_ge(cp_s, 2)
            sync.dma_start(out=out_v[:, 0], in_=sb_out[:, 0:1024]).then_inc(out_sem, 16)
            sync.wait_ge(cp_v, 4)
            sync.wait_ge(cp_s, 4)
            sync.dma_start(out=out_v[:, 1], in_=sb_out[:, 1024:2048]).then_inc(
                out_sem, 16
            )

        @block.tensor
        def _(tensor):
            tensor.wait_ge(dma_a, 16)
            tensor.matmul(
                pst[:, 0:NH], lhsT=P[:, 0:M], rhs=P[:, P0 : P0 + NH],
                start=True, stop=False,
            )
            tensor.wait_ge(dma_b, 16)
            tensor.matmul(
                pst[:, 0:NH], lhsT=P[:, M:WCOLS], rhs=P[:, P1 : P1 + NH],
                start=False, stop=True,
            ).then_inc(mm_sem, 1)
            tensor.matmul(
                pst[:, NW : NW + NH], lhsT=P[:, 0:M], rhs=P[:, P0 + NH : P0 + NW],
                start=True, stop=False,
            )
            tensor.wait_ge(dma_c, 16)
            tensor.matmul(
                pst[:, NW : NW + NH], lhsT=P[:, M:WCOLS], rhs=P[:, P1 + NH : P1 + NW],
                start=False, stop=True,
            ).then_inc(mm_sem, 1)

        @block.vector
        def _(vector):
            for h in range(2):
                vector.wait_ge(mm_sem, h + 1)
                for e in (0, 1):
                    dst, src = copy_ap(e, h)
                    vector.tensor_copy(dst, src).then_inc(cp_v, 1)

        @block.scalar
        def _(scalar):
            # Dummy tiny copy pulls ACT_TABLE_LOAD off the critical path.
            # Src is a preamble-initialized const tensor (no input dependency).
            ones = nc.const_aps.aps[(mybir.dt.bfloat16, 1.0)]
            scalar.copy(scratch[0:1, 0:1], ones[0:1, 0:1])
            for h in range(2):
                scalar.wait_ge(mm_sem, h + 1)
                for e in (2, 3):
                    dst, src = copy_ap(e, h)
                    scalar.copy(dst, src).then_inc(cp_s, 1)

    return nc


def make_in_maps(x, up_w, up_b, out_b):
    """Shard inputs: per-core block-diagonal im2col + packed weights."""
    import ml_dtypes

    bf16 = ml_dtypes.bfloat16
    x = np.asarray(x, dtype=np.float32)
    up_w = np.asarray(up_w, dtype=np.float32)
    up_b = np.asarray(up_b, dtype=np.float32)
    out_b = np.asarray(out_b, dtype=np.float32)

    xp = np.zeros((B, C, H + 2, W + 2), dtype=np.float32)
    xp[:, :, 1 : H + 1, 1 : W + 1] = x

    # Stationary weights, shared across cores.
    # lhsT_t[14g + kappa, m] = [g == g(m)] * w(tau=14t+kappa; m), block-diag.
    wb = np.zeros((KK, WCOLS), dtype=np.float32)
    for e in range(4):
        dr, dc = e // 2, e % 2
        for c in range(C):
            o = c * 4 + dr * 2 + dc
            for g in range(G):
                b2, rb = divmod(g, 4)
                m = e * 32 + 12 * b2 + 4 * c + rb
                for tau in range(27):
                    c2, kh, kw = tau // 9, (tau // 3) % 3, tau % 3
                    t, kappa = divmod(tau, KP)
                    wb[KP * g + kappa, M * t + m] = up_w[o, c2, kh, kw]
                # tau=27 (t=1, kappa=13): ones-row bias
                wb[KP * g + 13, M + m] = up_b[o] + out_b[c]

    in_maps = []
    for i in range(N_CORES):
        xcol = np.empty((KK, XW), dtype=np.float32)
        xcol[:, :WCOLS] = wb
        pat = xcol[:, WCOLS:].reshape(KK, 2, 4, W)  # [row, t, n, w]
        for g in range(G):
            b, rb = divmod(g, 4)
            r0 = RH * i + 4 * rb
            for kappa in range(KP):
                for t in range(2):
                    tau = KP * t + kappa
                    if tau == 27:
                        pat[KP * g + kappa, t] = 1.0
                    else:
                        c, kh, kw = tau // 9, (tau // 3) % 3, tau % 3
                        pat[KP * g + kappa, t] = xp[
                            b, c, r0 + kh : r0 + kh + 4, kw : kw + W
                        ]
        in_maps.append({"xcol": xcol.astype(bf16)})
    return in_maps


def kernel(x, up_w, up_b, in_w, in_b, adder_w, out_w, out_b):
    nc = build_graph()
    in_maps = make_in_maps(x, up_w, up_b, out_b)
    res = run_bass_kernel_spmd(nc, in_maps, core_ids=list(range(N_CORES)))
    slabs = [np.asarray(res.results[i]["out"]) for i in range(N_CORES)]
    return np.concatenate(slabs, axis=2).astype(np.float32)


# revision 15
# speedup vs baseline: 1.5103x; 1.0363x over previous
"""AdderVDSR kernel for 8 TRN2 NeuronCores.

Mathematical collapse: every AdderNet block computes
    relu(-sum_{c,kh,kw} |patch - w|)
and the inner sum of 576 absolute values of continuous random quantities is
strictly positive, so each block outputs exactly 0 in fp32.  After the first
adder layer the hidden state is identically zero and stays zero, so

    reference(x, ...) == pixel_shuffle(conv3(x, up_w, up_b), 2) + out_b

bit-exactly.  The kernel therefore only computes the 3->12 channel 3x3
up-conv, the pixel shuffle, and the two bias adds.

Distribution: data-parallel over H; core i computes pre-shuffle rows
[16i, 16i+16) -> output rows [32i, 32i+32).

Device formulation: block-diagonal im2col GEMM over G=8 groups (b x four
4-row blocks).  K = 8 groups x 14 taps = 112, contracted in two accumulating
passes (taps 0..13, then 14..26 + ones/bias row); M = 128 PSUM partitions =
4 shuffle phases (dr,dc) at 32-aligned bases + 8c + g; N = 512 pixels in one
PSUM bank, pipelined as two 256-column halves (4 matmuls).  Pixel-shuffle
interleave copies (24 lanes, phase x col-half, split DVE/ACT) land in a
24-partition SBUF layout whose partitions are 8 KiB DRAM-contiguous slab
rows, so each output DMA fans out across all DMA engines.

The NEFF epilogue (walrus' ~7us all-semaphore reset sweep, gated on the
block-exit barrier) dominates at this scale, so no engine waits for output
DMA completion: the sweep overlaps the output drain.
"""

import numpy as np

import concourse.bass as bass
import concourse.mybir as mybir
from concourse.bass_utils import run_bass_kernel_spmd

N_CORES = 8
B, C, H, W = 2, 3, 128, 128
RH = H // N_CORES          # 16 pre-shuffle rows per core
G = 8                      # groups: (b, 4-row block)
KP = 14                    # taps per pass (27 taps + ones row = 2x14)
KK = G * KP                # 112 matmul contraction rows
M = 128                    # psum partitions: 32*(dr,dc) + 8c + g, 32-aligned
NW = 512                   # pixels per group: 4 rows x 128 w
NH = NW // 2               # matmul column-half
WCOLS = 2 * M              # two passes' stationary weights
XW = WCOLS + 2 * NW        # 1280 total xcol columns

_f32 = mybir.dt.float32
_bf16 = mybir.dt.bfloat16

# rhs column blocks: [weights | p0c0 | p0c1 | p1c0 | p1c1]
P0 = WCOLS
P1 = WCOLS + NW
# input chunks: A covers weights + all of pass 0, B covers pass 1
CHA = WCOLS + NW


def build_graph():
    nc = bass.Bass()
    xcol = nc.declare_dram_parameter("xcol", [KK, XW], _bf16, isOutput=False)
    out = nc.declare_dram_parameter("out", [B, C, 2 * RH, 2 * W], _f32, isOutput=True)

    with (
        nc.sbuf_tensor([KK, XW], _bf16) as P,
        nc.sbuf_tensor([24, 2048], _f32) as sb_out,
        nc.sbuf_tensor([1, 16], _bf16) as scratch,
        nc.psum_tensor([M, 2 * NW], _f32) as pst,
        nc.semaphore("dma_a") as dma_a,
        nc.semaphore("dma_b") as dma_b,
        nc.semaphore("mm_sem") as mm_sem,
        nc.semaphore("cp_v") as cp_v,
        nc.semaphore("cp_s") as cp_s,
        nc.semaphore("out_sem") as out_sem,
        nc.Block() as block,
    ):
        # Output DRAM view: partition q = 12b + 4c + rb is the contiguous
        # 8 KiB slab out[b, c, 8*rb : 8*rb+8, :]; h splits it into 4 KiB
        # row-halves matching psum column-halves.
        out_v = out.rearrange(
            "b c (rb h rows) w -> (b c rb) h (rows w)", rb=4, h=2, rows=4
        )

        def copy_ap(e, h):
            dr, dc = e // 2, e % 2
            src = pst[32 * e : 32 * e + 24, NW * h : NW * h + NH].rearrange(
                "p (n w) -> p n w", n=2
            )
            dst = sb_out.rearrange(
                "q (n dr w dc) -> q n dr w dc", n=4, dr=2, w=128, dc=2
            )[:, 2 * h : 2 * h + 2, dr, :, dc]
            return dst, src

        @block.sync
        def _(sync):
            sync.dma_start(out=P[:, :CHA], in_=xcol[:, :CHA]).then_inc(dma_a, 16)
            sync.dma_start(out=P[:, CHA:], in_=xcol[:, CHA:]).then_inc(dma_b, 16)
            # Row-half output DMAs chase the interleave copies; nothing waits
            # for their completion -- the NEFF epilogue overlaps the drain.
            sync.wait_ge(cp_v, 2)
            sync.wait_ge(cp_s, 2)
            sync.dma_start(out=out_v[:, 0], in_=sb_out[:, 0:1024]).then_inc(out_sem, 16)
            sync.wait_ge(cp_v, 4)
            sync.wait_ge(cp_s, 4)
            sync.dma_start(out=out_v[:, 1], in_=sb_out[:, 1024:2048]).then_inc(
                out_sem, 16
            )

        @block.tensor
        def _(tensor):
            tensor.wait_ge(dma_a, 16)
            tensor.matmul(
                pst[:, 0:NH], lhsT=P[:, 0:M], rhs=P[:, P0 : P0 + NH],
                start=True, stop=False,
            )
            tensor.wait_ge(dma_b, 16)
            tensor.matmul(
                pst[:, 0:NH], lhsT=P[:, M:WCOLS], rhs=P[:, P1 : P1 + NH],
                start=False, stop=True,
            ).then_inc(mm_sem, 1)
            tensor.matmul(
                pst[:, NW : NW + NH], lhsT=P[:, 0:M], rhs=P[:, P0 + NH : P0 + NW],
                start=True, stop=False,
            )
            tensor.matmul(
                pst[:, NW : NW + NH], lhsT=P[:, M:WCOLS], rhs=P[:, P1 + NH : P1 + NW],
                start=False, stop=True,
            ).then_inc(mm_sem, 1)

        @block.vector
        def _(vector):
            for h in range(2):
                vector.wait_ge(mm_sem, h + 1)
                for e in (0, 1):
                    dst, src = copy_ap(e, h)
                    vector.tensor_copy(dst, src).then_inc(cp_v, 1)

        @block.scalar
        def _(scalar):
            # Dummy tiny copy pulls ACT_TABLE_LOAD off the critical path.
            # Src is a preamble-initialized const tensor (no input dependency).
            ones = nc.const_aps.aps[(mybir.dt.bfloat16, 1.0)]
            scalar.copy(scratch[0:1, 0:1], ones[0:1, 0:1])
            for h in range(2):
                scalar.wait_ge(mm_sem, h + 1)
                for e in (2, 3):
                    dst, src = copy_ap(e, h)
                    scalar.copy(dst, src).then_inc(cp_s, 1)

    return nc


def make_in_maps(x, up_w, up_b, out_b):
    """Shard inputs: per-core block-diagonal im2col + packed weights."""
    import ml_dtypes

    bf16 = ml_dtypes.bfloat16
    x = np.asarray(x, dtype=np.float32)
    up_w = np.asarray(up_w, dtype=np.float32)
    up_b = np.asarray(up_b, dtype=np.float32)
    out_b = np.asarray(out_b, dtype=np.float32)

    xp = np.zeros((B, C, H + 2, W + 2), dtype=np.float32)
    xp[:, :, 1 : H + 1, 1 : W + 1] = x

    # Stationary weights, shared across cores.
    # lhsT_t[14g + kappa, m] = [g == g(m)] * w(tau=14t+kappa; m), block-diag.
    wb = np.zeros((KK, WCOLS), dtype=np.float32)
    for e in range(4):
        dr, dc = e // 2, e % 2
        for c in range(C):
            o = c * 4 + dr * 2 + dc
            for g in range(G):
                b2, rb = divmod(g, 4)
                m = e * 32 + 12 * b2 + 4 * c + rb
                for tau in range(27):
                    c2, kh, kw = tau // 9, (tau // 3) % 3, tau % 3
                    t, kappa = divmod(tau, KP)
                    wb[KP * g + kappa, M * t + m] = up_w[o, c2, kh, kw]
                # tau=27 (t=1, kappa=13): ones-row bias
                wb[KP * g + 13, M + m] = up_b[o] + out_b[c]

    in_maps = []
    for i in range(N_CORES):
        xcol = np.empty((KK, XW), dtype=np.float32)
        xcol[:, :WCOLS] = wb
        pat = xcol[:, WCOLS:].reshape(KK, 2, 4, W)  # [row, t, n, w]
        for g in range(G):
            b, rb = divmod(g, 4)
            r0 = RH * i + 4 * rb
            for kappa in range(KP):
                for t in range(2):
                    tau = KP * t + kappa
                    if tau == 27:
                        pat[KP * g + kappa, t] = 1.0
                    else:
                        c, kh, kw = tau // 9, (tau // 3) % 3, tau % 3
                        pat[KP * g + kappa, t] = xp[
                            b, c, r0 + kh : r0 + kh + 4, kw : kw + W
                        ]
        in_maps.append({"xcol": xcol.astype(bf16)})
    return in_maps


def kernel(x, up_w, up_b, in_w, in_b, adder_w, out_w, out_b):
    nc = build_graph()
    in_maps = make_in_maps(x, up_w, up_b, out_b)
    res = run_bass_kernel_spmd(nc, in_maps, core_ids=list(range(N_CORES)))
    slabs = [np.asarray(res.results[i]["out"]) for i in range(N_CORES)]
    return np.concatenate(slabs, axis=2).astype(np.float32)


# revision 17
# speedup vs baseline: 1.5498x; 1.0262x over previous
"""AdderVDSR kernel for 8 TRN2 NeuronCores.

Mathematical collapse: every AdderNet block computes
    relu(-sum_{c,kh,kw} |patch - w|)
and the inner sum of 576 absolute values of continuous random quantities is
strictly positive, so each block outputs exactly 0 in fp32.  After the first
adder layer the hidden state is identically zero and stays zero, so

    reference(x, ...) == pixel_shuffle(conv3(x, up_w, up_b), 2) + out_b

bit-exactly.  The kernel therefore only computes the 3->12 channel 3x3
up-conv, the pixel shuffle, and the two bias adds.

Distribution: data-parallel over H; core i computes pre-shuffle rows
[16i, 16i+16) -> output rows [32i, 32i+32).

Device formulation: block-diagonal im2col GEMM over G=8 groups (b x four
4-row blocks).  K = 8 groups x 14 taps = 112, contracted in two accumulating
passes (taps 0..13, then 14..26 + ones/bias row); M = 128 PSUM partitions =
4 shuffle phases (dr,dc) at 32-aligned bases + 8c + g; N = 512 pixels in one
PSUM bank, pipelined as two 256-column halves (4 matmuls).  Pixel-shuffle
interleave copies (24 lanes, phase x col-half, split DVE/ACT) land in a
24-partition SBUF layout whose partitions are 8 KiB DRAM-contiguous slab
rows, so each output DMA fans out across all DMA engines.

The NEFF epilogue (walrus' ~7us all-semaphore reset sweep, gated on the
block-exit barrier) dominates at this scale, so no engine waits for output
DMA completion: the sweep overlaps the output drain.
"""

import numpy as np

import concourse.bass as bass
import concourse.mybir as mybir
from concourse.bass_utils import run_bass_kernel_spmd

N_CORES = 8
B, C, H, W = 2, 3, 128, 128
RH = H // N_CORES          # 16 pre-shuffle rows per core
G = 8                      # groups: (b, 4-row block)
KP = 14                    # taps per pass (27 taps + ones row = 2x14)
KK = G * KP                # 112 matmul contraction rows
M = 128                    # psum partitions: 32*(dr,dc) + 8c + g, 32-aligned
NW = 512                   # pixels per group: 4 rows x 128 w
NH = NW // 2               # matmul column-half
WCOLS = 2 * M              # two passes' stationary weights
XW = WCOLS + 2 * NW        # 1280 total xcol columns

_f32 = mybir.dt.float32
_bf16 = mybir.dt.bfloat16

# rhs column blocks: [weights | p0c0 | p1c0 | p0c1 | p1c1] -- chunk A is
# everything column-half 0 needs (both passes), so the h0 matmul pair and the
# first interleave copies start before chunk B lands.
R1 = WCOLS            # p0c0
R2 = WCOLS + NH       # p1c0
R3 = WCOLS + NW       # p0c1
R4 = WCOLS + NW + NH  # p1c1
CHA = WCOLS + NW


def build_graph():
    nc = bass.Bass()
    xcol = nc.declare_dram_parameter("xcol", [KK, XW], _bf16, isOutput=False)
    out = nc.declare_dram_parameter("out", [B, C, 2 * RH, 2 * W], _f32, isOutput=True)

    with (
        nc.sbuf_tensor([KK, XW], _bf16) as P,
        nc.sbuf_tensor([24, 2048], _f32) as sb_out,
        nc.sbuf_tensor([1, 16], _bf16) as scratch,
        nc.psum_tensor([M, 2 * NW], _f32) as pst,
        nc.psum_tensor([1, 16], _f32) as warm,
        nc.semaphore("dma_a") as dma_a,
        nc.semaphore("dma_b") as dma_b,
        nc.semaphore("mm_sem") as mm_sem,
        nc.semaphore("cp_v") as cp_v,
        nc.semaphore("cp_s") as cp_s,
        nc.semaphore("out_sem") as out_sem,
        nc.Block() as block,
    ):
        # Output DRAM view: partition q = 12b + 4c + rb is the contiguous
        # 8 KiB slab out[b, c, 8*rb : 8*rb+8, :]; h splits it into 4 KiB
        # row-halves matching psum column-halves.
        out_v = out.rearrange(
            "b c (rb h rows) w -> (b c rb) h (rows w)", rb=4, h=2, rows=4
        )

        def copy_ap(e, h):
            dr, dc = e // 2, e % 2
            src = pst[32 * e : 32 * e + 24, NW * h : NW * h + NH].rearrange(
                "p (n w) -> p n w", n=2
            )
            dst = sb_out.rearrange(
                "q (n dr w dc) -> q n dr w dc", n=4, dr=2, w=128, dc=2
            )[:, 2 * h : 2 * h + 2, dr, :, dc]
            return dst, src

        @block.sync
        def _(sync):
            sync.dma_start(out=P[:, :CHA], in_=xcol[:, :CHA]).then_inc(dma_a, 16)
            sync.dma_start(out=P[:, CHA:], in_=xcol[:, CHA:]).then_inc(dma_b, 16)
            # Row-half output DMAs chase the interleave copies; nothing waits
            # for their completion -- the NEFF epilogue overlaps the drain.
            sync.wait_ge(cp_v, 2)
            sync.wait_ge(cp_s, 2)
            sync.dma_start(out=out_v[:, 0], in_=sb_out[:, 0:1024]).then_inc(out_sem, 16)
            sync.wait_ge(cp_v, 4)
            sync.wait_ge(cp_s, 4)
            sync.dma_start(out=out_v[:, 1], in_=sb_out[:, 1024:2048]).then_inc(
                out_sem, 16
            )

        @block.tensor
        def _(tensor):
            # PE clock ramps 1.2->2.4 GHz after ~3.4us of activity; tiny
            # dummy matmuls on a const tensor during the input drain warm it
            # so the real matmuls run at the fast clock.
            ones = nc.const_aps.aps[(mybir.dt.bfloat16, 1.0)]
            for _ in range(12):
                tensor.matmul(
                    warm[0:1, 0:1], lhsT=ones[0:128, 0:1], rhs=ones[0:128, 0:1],
                    start=True, stop=True,
                )
            tensor.wait_ge(dma_a, 16)
            tensor.matmul(
                pst[:, 0:NH], lhsT=P[:, 0:M], rhs=P[:, R1 : R1 + NH],
                start=True, stop=False,
            )
            tensor.matmul(
                pst[:, 0:NH], lhsT=P[:, M:WCOLS], rhs=P[:, R2 : R2 + NH],
                start=False, stop=True,
            ).then_inc(mm_sem, 1)
            tensor.wait_ge(dma_b, 16)
            tensor.matmul(
                pst[:, NW : NW + NH], lhsT=P[:, 0:M], rhs=P[:, R3 : R3 + NH],
                start=True, stop=False,
            )
            tensor.matmul(
                pst[:, NW : NW + NH], lhsT=P[:, M:WCOLS], rhs=P[:, R4 : R4 + NH],
                start=False, stop=True,
            ).then_inc(mm_sem, 1)

        @block.vector
        def _(vector):
            for h in range(2):
                vector.wait_ge(mm_sem, h + 1)
                for e in (0, 1):
                    dst, src = copy_ap(e, h)
                    vector.tensor_copy(dst, src).then_inc(cp_v, 1)

        @block.scalar
        def _(scalar):
            # Dummy tiny copy pulls ACT_TABLE_LOAD off the critical path.
            # Src is a preamble-initialized const tensor (no input dependency).
            ones = nc.const_aps.aps[(mybir.dt.bfloat16, 1.0)]
            scalar.copy(scratch[0:1, 0:1], ones[0:1, 0:1])
            for h in range(2):
                scalar.wait_ge(mm_sem, h + 1)
                for e in (2, 3):
                    dst, src = copy_ap(e, h)
                    scalar.copy(dst, src).then_inc(cp_s, 1)

    return nc


def make_in_maps(x, up_w, up_b, out_b):
    """Shard inputs: per-core block-diagonal im2col + packed weights."""
    import ml_dtypes

    bf16 = ml_dtypes.bfloat16
    x = np.asarray(x, dtype=np.float32)
    up_w = np.asarray(up_w, dtype=np.float32)
    up_b = np.asarray(up_b, dtype=np.float32)
    out_b = np.asarray(out_b, dtype=np.float32)

    xp = np.zeros((B, C, H + 2, W + 2), dtype=np.float32)
    xp[:, :, 1 : H + 1, 1 : W + 1] = x

    # Stationary weights, shared across cores.
    # lhsT_t[14g + kappa, m] = [g == g(m)] * w(tau=14t+kappa; m), block-diag.
    wb = np.zeros((KK, WCOLS), dtype=np.float32)
    for e in range(4):
        dr, dc = e // 2, e % 2
        for c in range(C):
            o = c * 4 + dr * 2 + dc
            for g in range(G):
                b2, rb = divmod(g, 4)
                m = e * 32 + 12 * b2 + 4 * c + rb
                for tau in range(27):
                    c2, kh, kw = tau // 9, (tau // 3) % 3, tau % 3
                    t, kappa = divmod(tau, KP)
                    wb[KP * g + kappa, M * t + m] = up_w[o, c2, kh, kw]
                # tau=27 (t=1, kappa=13): ones-row bias
                wb[KP * g + 13, M + m] = up_b[o] + out_b[c]

    in_maps = []
    for i in range(N_CORES):
        xcol = np.empty((KK, XW), dtype=np.float32)
        xcol[:, :WCOLS] = wb
        pat = np.empty((KK, 2, 4, W), dtype=np.float32)  # [row, t, n, w]
        for g in range(G):
            b, rb = divmod(g, 4)
            r0 = RH * i + 4 * rb
            for kappa in range(KP):
                for t in range(2):
                    tau = KP * t + kappa
                    if tau == 27:
                        pat[KP * g + kappa, t] = 1.0
                    else:
                        c, kh, kw = tau // 9, (tau // 3) % 3, tau % 3
                        pat[KP * g + kappa, t] = xp[
                            b, c, r0 + kh : r0 + kh + 4, kw : kw + W
                        ]
        # column order (h, t, n', w): chunk A = both passes of n-half 0
        xcol[:, WCOLS:] = (
            pat.reshape(KK, 2, 2, 2, W).transpose(0, 2, 1, 3, 4).reshape(KK, 8 * W)
        )
        in_maps.append({"xcol": xcol.astype(bf16)})
    return in_maps


def kernel(x, up_w, up_b, in_w, in_b, adder_w, out_w, out_b):
    nc = build_graph()
    in_maps = make_in_maps(x, up_w, up_b, out_b)
    res = run_bass_kernel_spmd(nc, in_maps, core_ids=list(range(N_CORES)))
    slabs = [np.asarray(res.results[i]["out"]) for i in range(N_CORES)]
    return np.concatenate(slabs, axis=2).astype(np.float32)
